# revision 19
# baseline (speedup 1.0000x reference)
"""Block-diagonal linear layer (BlockLinearLayer) on 8 Trainium2 NeuronCores.

Math: x [65536, 4096] -> view [B, 128 blocks, 32]; out[b,n,j] = sum_k x3[b,n,k]*W[n,j,k] + bias
   -> out [65536, 1024].

Strategy (data-parallel over batch, 8 cores x 8192 rows), bf16 edition:
- The kernel is HBM-bandwidth-bound (DMA active ~95% of exec in the f32
  version), so x streams to SBUF as bf16 and the output streams back as bf16,
  halving the dominant traffic: 67.1 MB in + 16.8 MB out per core. PSUM
  accumulation stays f32; l2 relative error ~2e-3 (gate is 2e-2).
- W is expanded on host into block-diagonal [128, 32] bf16 tiles per feature
  group g (4 blocks = 128 features -> 32 outputs), stored as wd [128, 1024].
  W is the *stationary* matmul operand; x is the moving operand at N=512.
- Host packs per-core x so each 2 MiB DMA is fully contiguous per partition
  (16 KiB runs) with the feature group on partitions:
  xq[q, s, p, gg*2048 + b] = x[2048*s + b, 512*q + 128*gg + p]  (bf16).
- Output lands transposed in PSUM ([32 outs, 512 batch] per matmul); four
  groups (one "quad" q) stack into the 128 PSUM partitions via col-tiling
  (tile_position=(0, 32*gg)). DVE adds per-partition bias while copying
  PSUM f32 -> SBUF bf16; 2 MiB contiguous stores write outT [1024, 8192]
  bf16 per core; host transposes/casts outT back (output is 4x smaller
  than the input, and only 2 MiB per quad).
- The PSUM->SBUF bias-copy is split between DVE (cols 0-1023, PSUM banks 0-1)
  and the ACT engine via Identity activation with per-partition bias (cols
  1024-2047, banks 2-3) — different banks, so the reads run in parallel and
  the copy stage that paces the end-of-kernel store drain is ~1.3 us instead
  of 2.4 us.
- Deep DMA lookahead (8 x-tile buffers, 4 output buffers) keeps the SDMA
  engines fed; loads own the sync HWDGE ring and stores own the scalar ring
  (mixing them head-of-line blocks loads behind DVE-gated stores), W/bias ride
  SWDGE. Stores go out per strip (0.5 MiB), and the final strip is tapered
  into 512-column chunks so the matmul -> copy -> store pipeline drains fast.
- Measured: ~219 us best / ~220 us median on idle HW (84.2 MB/core at ~3.05
  TB/s device aggregate = the HBM roofline; f32 same structure: 437 us).
"""

import os

import numpy as np
import ml_dtypes

BF16 = np.dtype(ml_dtypes.bfloat16)

BATCH = 65536
INPUT_SIZE = 4096
OUTPUT_SIZE = 1024
N_BLOCKS = 128
BLOCK = 32
OPB = 8  # outputs per block
NCORES = 8
BC = BATCH // NCORES  # 8192 rows per core
P = 128
NQ = 8  # quads (4 feature groups each -> 128 output rows)
NS = 4  # batch strips per core
SB = 2048  # strip batch size

LAST_EXEC_NS = None

_cached = None


def _build_program():
    import concourse.bass as bass
    import concourse.tile as tile
    from concourse import bacc, mybir
    from concourse.bass import ts

    f32 = mybir.dt.float32
    bf16 = mybir.dt.bfloat16
    nc = bacc.Bacc("TRN2", target_bir_lowering=False, debug=False, num_devices=NCORES)

    xq = nc.dram_tensor("xq", [NQ, NS, P, 4 * SB], bf16, kind="ExternalInput").ap()
    wd = nc.dram_tensor("wd", [P, OUTPUT_SIZE], bf16, kind="ExternalInput").ap()
    biasT = nc.dram_tensor("biasT", [P, NQ], f32, kind="ExternalInput").ap()
    outT = nc.dram_tensor("outT", [OUTPUT_SIZE, BC], bf16, kind="ExternalOutput").ap()
    outTv = outT.rearrange("(q p) m -> q p m", p=P)  # [8, 128, 8192]

    with tile.TileContext(nc) as tc:
        with (
            tc.tile_pool(name="xpool", bufs=8) as xpool,
            tc.tile_pool(name="wpool", bufs=1) as wpool,
            tc.tile_pool(name="bpool", bufs=1) as bpool,
            tc.tile_pool(name="opool", bufs=4) as opool,
            tc.tile_pool(name="pspool", bufs=2, space="PSUM") as pspool,
        ):
            # W + bias ride the SWDGE (gpsimd) queue so they never head-block
            # the two HWDGE rings that carry the bulk x/out traffic.
            wtile = wpool.tile([P, OUTPUT_SIZE], bf16)
            nc.gpsimd.dma_start(wtile[:], wd)
            btile = bpool.tile([P, NQ], f32)
            nc.gpsimd.dma_start(btile[:], biasT)

            for q in range(NQ):
                for s in range(NS):
                    it = q * NS + s
                    # Dedicated rings: every load on sync, every store on
                    # scalar. Mixing them on one sequencer head-of-line
                    # blocks loads behind stores that wait on DVE results
                    # (measured: 265 us mixed vs 222 us dedicated).
                    ldq = nc.sync
                    stq = nc.scalar
                    xt = xpool.tile([P, 4 * SB], bf16)
                    ldq.dma_start(xt[:], xq[q, s])
                    ps = pspool.tile([P, SB], f32)
                    last = it == NQ * NS - 1
                    ot = opool.tile([P, SB], bf16)
                    H = SB // 2
                    if not last:
                        for gg in range(4):
                            for h in range(4):
                                nc.tensor.matmul(
                                    ps[32 * gg : 32 * (gg + 1), ts(h, 512)],
                                    wtile[:, ts(4 * q + gg, BLOCK)],
                                    xt[:, SB * gg + 512 * h : SB * gg + 512 * (h + 1)],
                                    start=True,
                                    stop=True,
                                    tile_position=(0, 32 * gg),
                                )
                        # Split the PSUM->SBUF bias-copy across ACT and DVE
                        # (different PSUM bank pairs, legal in parallel):
                        # halves the per-strip copy latency that paces the
                        # end-of-kernel store drain. ACT op is emitted first
                        # so its sequencer isn't stalled behind the store
                        # dispatch that waits on DVE.
                        nc.scalar.activation(
                            out=ot[:, H:],
                            in_=ps[:, H:],
                            func=mybir.ActivationFunctionType.Identity,
                            bias=btile[:, q : q + 1],
                        )
                        nc.vector.tensor_scalar_add(
                            out=ot[:, :H],
                            in0=ps[:, :H],
                            scalar1=btile[:, q : q + 1],
                        )
                        stq.dma_start(outTv[q][:, ts(s, SB)], ot[:])
                    else:
                        # Taper the final strip: 512-column chunks (one PSUM
                        # bank each) alternate between DVE and ACT so two
                        # chunks drain concurrently, each chased by its own
                        # 128 KiB store.
                        for h in range(4):
                            for gg in range(4):
                                nc.tensor.matmul(
                                    ps[32 * gg : 32 * (gg + 1), ts(h, 512)],
                                    wtile[:, ts(4 * q + gg, BLOCK)],
                                    xt[:, SB * gg + 512 * h : SB * gg + 512 * (h + 1)],
                                    start=True,
                                    stop=True,
                                    tile_position=(0, 32 * gg),
                                )
                            if h % 2 == 0:
                                nc.vector.tensor_scalar_add(
                                    out=ot[:, ts(h, 512)],
                                    in0=ps[:, ts(h, 512)],
                                    scalar1=btile[:, q : q + 1],
                                )
                            else:
                                nc.scalar.activation(
                                    out=ot[:, ts(h, 512)],
                                    in_=ps[:, ts(h, 512)],
                                    func=mybir.ActivationFunctionType.Identity,
                                    bias=btile[:, q : q + 1],
                                )
                            stq.dma_start(
                                outTv[q][:, SB * s + 512 * h : SB * s + 512 * (h + 1)],
                                ot[:, ts(h, 512)],
                            )

    nc.compile()
    return nc


def _host_pack_w(W: np.ndarray) -> np.ndarray:
    # wd[f, 32g + o]: for f = 32qq + k, o = 8qq + j -> W[4g + qq, j, k]; else 0
    NGROUP = 32
    Wr = np.ascontiguousarray(W, dtype=np.float32).reshape(NGROUP, 4, OPB, BLOCK)
    Wd = np.zeros((NGROUP, P, BLOCK), dtype=np.float32)  # [g, f, o_local]
    for qq in range(4):
        Wd[:, BLOCK * qq : BLOCK * (qq + 1), OPB * qq : OPB * (qq + 1)] = Wr[
            :, qq
        ].transpose(0, 2, 1)
    return np.ascontiguousarray(
        Wd.transpose(1, 0, 2).reshape(P, OUTPUT_SIZE).astype(BF16)
    )


def _host_pack_x(xc16: np.ndarray) -> np.ndarray:
    # xq[q, s, p, gg*SB + b] = xc[SB*s + b, 512*q + 128*gg + p]  (bf16)
    x5 = xc16.reshape(NS, SB, NQ, 4, P)  # [s, b, q, gg, p]
    return np.ascontiguousarray(x5.transpose(2, 0, 4, 3, 1)).reshape(NQ, NS, P, 4 * SB)


def kernel(x: np.ndarray, W: np.ndarray, b: np.ndarray) -> np.ndarray:
    global LAST_EXEC_NS, _cached
    from concourse.bass_utils import run_bass_kernel_spmd

    x16 = np.asarray(x, dtype=np.float32).astype(BF16)
    wd = _host_pack_w(W)
    bT = np.ascontiguousarray(
        np.asarray(b, dtype=np.float32).reshape(NQ, P).T
    )  # [128, 8]

    if _cached is None:
        _cached = _build_program()
    nc = _cached

    in_maps = []
    for i in range(NCORES):
        xc = x16[i * BC : (i + 1) * BC]
        in_maps.append({"xq": _host_pack_x(xc), "wd": wd, "biasT": bT})

    trace = bool(os.environ.get("BLK_TRACE"))
    if trace:
        try:
            import ntff_shim  # noqa: F401
        except ImportError:
            trace = False
    if not trace:
        # If BASS_TRACE is set in the environment, bass_utils would import
        # antenv.axon_hooks (absent on this image) and crash. Register a stub
        # so it degrades to "hook isn't registered" and runs untraced. Only
        # stub when the real module genuinely can't be imported, so an
        # environment that does provide it keeps its own tracing intact.
        import sys
        import types

        if "antenv.axon_hooks" not in sys.modules:
            try:
                import antenv.axon_hooks  # noqa: F401
            except Exception:
                stub = types.ModuleType("antenv.axon_hooks")
                stub.get_axon_ntff_profile_hook = lambda: None
                stub.set_axon_ntff_profile_hook = lambda h: None
                sys.modules["antenv.axon_hooks"] = stub
    res = run_bass_kernel_spmd(nc, in_maps, core_ids=list(range(NCORES)), trace=trace)
    LAST_EXEC_NS = res.exec_time_ns

    out = np.empty((BATCH, OUTPUT_SIZE), dtype=np.float32)
    for i in range(NCORES):
        out[i * BC : (i + 1) * BC] = res.results[i]["outT"].T.astype(np.float32)
    return out
